# revision 27
# baseline (speedup 1.0000x reference)
"""Block-tensorized linear (TT-factored block linear) on 8 Trainium2 NeuronCores.

Problem (hardcoded shapes):
    x:    (4, 2048, 4096) fp32   -> 8192 tokens, 4096 features
    U:    (4, 4, 1024, 256) fp32 (rows, cols, block, rank)
    V:    (4, 4, 256, 1024) fp32 (rows, cols, rank, block)
    bias: (4, 1024) fp32
    y[t, o*1024+m] = sum_c sum_r (sum_v x[t, c*1024+v] V[o,c,r,v]) U[o,c,m,r] + bias[o,m]

Sharding: 2-way tensor parallel over output row-block pairs (cores 0-3 take
o in {0,1}, cores 4-7 take o in {2,3}) x 4-way data parallel over tokens
(2048 tokens per core). Each core keeps its transposed U/V resident in SBUF
and streams its token shard in 2 chunks of 1024 tokens.

All operands are bf16 (cast on host): the PE runs bf16 at the same 1 cycle/row
as f32r, but every DMA byte halves, which keeps chunk-0's front-loaded demand
(x chunk + all of V^T + all of U^T) under the ~330 GB/s HBM ceiling, and bf16
LDWEIGHTS (~100ns) hides fully under the 213ns matmuls. Rel err ~4e-3 vs the
2e-2 gate. 1024-token chunks amortize the weight loads over twice the compute
window of 512-token chunks, which removes most chunk-0 DMA-wait stalls.

A short burst of dummy matmuls on a zeroed scratch tile runs during the ~8us
framework preamble + first-DMA window so the PE's HAM clock gate is already
at 2.4 GHz (it needs ~3.4us of sustained busy) when the first real matmul
issues; otherwise the first ~6 matmuls run at 1.2 GHz.

Stage 1 streams vj (the contraction) OUTER over four concurrent PSUM
accumulation groups (o x rj) per 512-token half, so the HBM demand is flat
from the first matmul. Engine split: TensorE does both matmul stages back to
back; VectorE rounds stage-1 PSUM to bf16 SBUF (CAST); ScalarE evicts stage-2
PSUM to bf16 SBUF. DMA queues: sync/HWDGE carries all weight loads (and
last-chunk y stores), scalar/HWDGE carries all x loads (first pieces sized
and ordered by first use), gpsimd/SWDGE carries chunk-0 y stores so a
backed-up store can never block a load ring. Bias is added on the host.
"""

import numpy as np
from contextlib import ExitStack

NCORES = 8
TOK = 8192            # total tokens
D = 4096              # features
NB = 4                # num row/col blocks
BS = 1024             # block size
R = 256               # TT rank
TQ = TOK // 4         # tokens per core (2048)
TC = 1024             # token chunk
NCHUNK = TQ // TC     # 2 chunks

_CACHE = {}


def _build_nc():
    if "nc" in _CACHE:
        return _CACHE["nc"]

    import concourse.bacc as bacc
    import concourse.tile as tile
    import concourse.mybir as mybir

    dt = mybir.dt
    bf16 = dt.bfloat16

    nc = bacc.Bacc("TRN2", target_bir_lowering=False, debug=False)

    # x shard, host-tiled: [chunk, c, partition(v%128), vj*TC + t]
    xh_d = nc.dram_tensor("xh", [NCHUNK, NB, 128, 8 * TC], bf16, kind="ExternalInput").ap()
    # V^T per (o_loc, c) block: [b, p(v%128), vj*256 + r]
    vt_d = nc.dram_tensor("vt", [8, 128, 2048], bf16, kind="ExternalInput").ap()
    # U^T per (o_loc, c) block: [b, p(r%128), rj*1024 + m]
    ut_d = nc.dram_tensor("ut", [8, 128, 2048], bf16, kind="ExternalInput").ap()
    # output shard: [2048 tokens, o_loc*1024 + m] (bias + f32 upcast host-side)
    y_d = nc.dram_tensor("y", [TQ, 2048], bf16, kind="ExternalOutput").ap()

    GROUPS = ((0, 0), (0, 1), (1, 0), (1, 1))  # (o, rj)

    with tile.TileContext(nc) as tcx, ExitStack() as ctx:
        wpool = ctx.enter_context(tcx.tile_pool(name="w", bufs=1))
        xpool = ctx.enter_context(tcx.tile_pool(name="xp", bufs=4))
        zpool = ctx.enter_context(tcx.tile_pool(name="zp", bufs=1))
        ypool = ctx.enter_context(tcx.tile_pool(name="yp", bufs=7))
        warm_pool = ctx.enter_context(tcx.tile_pool(name="wm", bufs=1))
        # 5+3 of the 8 PSUM banks: the 5th zps slot gives each th-set's first
        # accumulation group an extra ~850ns of slack on the bank-recycle WAR
        # (stop -> cast -> reuse) at th-seams; stage-2's evictions have ~4us
        # of slack, so yps can spare a bank
        zps_pool = ctx.enter_context(tcx.tile_pool(name="zps", bufs=5, space="PSUM"))
        yps_pool = ctx.enter_context(tcx.tile_pool(name="yps", bufs=3, space="PSUM"))

        # ---- PE warm-up: ~3.4us of dummy matmuls on zeroed scratch so the
        # HAM clock gate reaches 2.4 GHz before the first real matmul ----
        ws = warm_pool.tile([128, 512], bf16, tag="warm")
        nc.gpsimd.memset(ws[:], 0.0)
        wps = zps_pool.tile([128, 512], dt.float32, tag="zps", name="warm_ps")
        for _ in range(10):
            nc.tensor.matmul(wps[:], ws[:, 0:128], ws[:], start=True, stop=True)

        vtt = [None] * 8
        utt = [None] * 8
        xtiles = {}

        def load_vt_pair(c):
            # medium pieces (256 KB+, >=2 KB per partition row): small pieces
            # throttle the DGE ring on dispatch/descriptor overhead, whole
            # blocks serialize the startup critical path; c=0 is split so the
            # first matmul's deps land early, later blocks go whole
            ta = wpool.tile([128, 2048], bf16, tag=f"vt{c}")
            tb = wpool.tile([128, 2048], bf16, tag=f"vt{4 + c}")
            if c == 0:
                for s in (slice(0, 512), slice(512, 1024), slice(1024, 2048)):
                    nc.sync.dma_start(ta[:, s], vt_d[c][:, s])
                    nc.sync.dma_start(tb[:, s], vt_d[4 + c][:, s])
            else:
                nc.sync.dma_start(ta[:], vt_d[c])
                nc.sync.dma_start(tb[:], vt_d[4 + c])
            vtt[c], vtt[4 + c] = ta, tb

        def emit_stage1(tc_i):
            zsb = {}
            for c in range(NB):
                if tc_i == 0:
                    load_vt_pair(c)
                    xc = xpool.tile([128, 8 * TC], bf16, tag="xc")
                    if c == 0:
                        # the early HBM window is bandwidth-capped no matter
                        # how many rings are active, so pieces must arrive in
                        # exact consumption order on one ring, finest first
                        pieces = (slice(0, 512), slice(512, 1024),
                                  slice(1024, 2048), slice(2048, 4096),
                                  slice(4096, 6144), slice(6144, 8192))
                    elif c == 1:
                        pieces = (slice(0, 2048), slice(2048, 4096),
                                  slice(4096, 6144), slice(6144, 8192))
                    else:
                        pieces = (slice(0, 8192),)
                    for s in pieces:
                        nc.scalar.dma_start(xc[:, s], xh_d[tc_i, c, :, s])
                else:
                    xc = xtiles.pop((tc_i, c))
                for th in range(2):
                    zps = {}
                    for g in GROUPS:
                        zps[g] = zps_pool.tile([128, 512], dt.float32, tag="zps", name="zps")
                    for vj in range(8):
                        for o, rj in GROUPS:
                            b = o * 4 + c
                            nc.tensor.matmul(
                                zps[(o, rj)][:],
                                vtt[b][:, vj * 256 + rj * 128 : vj * 256 + rj * 128 + 128],
                                xc[:, th * 4096 + vj * 512 : th * 4096 + vj * 512 + 512],
                                start=(vj == 0),
                                stop=(vj == 7),
                            )
                    for gi, (o, rj) in enumerate(GROUPS):
                        b = o * 4 + c
                        if th == 0:
                            zsb[(b, rj)] = zpool.tile(
                                [128, TC], bf16, tag=f"z{b}_{rj}", name=f"z{b}_{rj}"
                            )
                        # alternate cast engines so two PSUM banks recycle in
                        # parallel at the th-seam (different banks: legal)
                        dst = zsb[(b, rj)][:, th * 512 : (th + 1) * 512]
                        if gi % 2 == 0:
                            nc.vector.tensor_copy(dst, zps[(o, rj)][:])
                        else:
                            nc.scalar.copy(dst, zps[(o, rj)][:])
            return zsb

        def emit_stage2(tc_i, zsb):
            ev = 0
            for o in range(2):
                for mc in range(2):
                    for tt in range(TC // 128):
                        last_group = (
                            tc_i == NCHUNK - 1 and o == 1 and mc == 1 and tt == TC // 128 - 1
                        )
                        t0 = tc_i * TC + tt * 128
                        mu0 = mc * 512           # m offset within a U^T rj half
                        m0 = o * 1024 + mc * 512  # m offset within the y row
                        # the very last group runs as two 256-wide halves so
                        # the final eviction+store chain after the last matmul
                        # is half as long
                        halves = ((0, 256), (256, 512)) if last_group else ((0, 512),)
                        for hi, (mlo, mhi) in enumerate(halves):
                            mw = mhi - mlo
                            yps = yps_pool.tile([128, mw], dt.float32, tag="yps", name="yps")
                            k = 0
                            for c in range(NB):
                                b = o * 4 + c
                                for rj in range(2):
                                    nc.tensor.matmul(
                                        yps[:],
                                        zsb[(b, rj)][:, tt * 128 : (tt + 1) * 128],
                                        utt[b][:, rj * 1024 + mu0 + mlo : rj * 1024 + mu0 + mhi],
                                        start=(k == 0),
                                        stop=(k == 7),
                                    )
                                    k += 1
                            ysb = ypool.tile([128, mw], bf16, tag="ysb", name="ysb")
                            if last_group and hi == 1:
                                nc.vector.tensor_copy(ysb[:], yps[:])
                            else:
                                nc.scalar.copy(ysb[:], yps[:])
                            # y stores go out on the SWDGE path so they never
                            # queue ahead of loads in the HWDGE rings; the last
                            # chunk switches to the (by then idle) sync ring to
                            # skip the SWDGE drain at the tail
                            eng = nc.sync if tc_i == NCHUNK - 1 else nc.gpsimd
                            eng.dma_start(
                                y_d[t0 : t0 + 128, m0 + mlo : m0 + mhi], ysb[:]
                            )
                        ev += 1
                        if ev == 2 and tc_i < NCHUNK - 1:
                            # prefetch next chunk's x now: late enough that the
                            # transfers don't contend with chunk-0's U^T loads,
                            # early enough to land before stage 1 of chunk i+1
                            for c in range(NB):
                                xt = xpool.tile([128, 8 * TC], bf16, tag="xc")
                                nc.scalar.dma_start(xt[:], xh_d[tc_i + 1, c])
                                xtiles[(tc_i + 1, c)] = xt

        for tc_i in range(NCHUNK):
            zsb = emit_stage1(tc_i)
            if tc_i == 0:
                # U^T as whole blocks in first-use order, on the SCALAR ring
                # behind chunk-0's x loads: the ring's FIFO order keeps these
                # (first needed at ~67us) from stealing HBM bandwidth from the
                # x blocks stage 1 is actively consuming
                for b in range(8):
                    utt[b] = wpool.tile([128, 2048], bf16, tag=f"ut{b}", name=f"ut{b}")
                    nc.scalar.dma_start(utt[b][:], ut_d[b])
            emit_stage2(tc_i, zsb)

    nc.compile()
    _CACHE["nc"] = nc
    return nc


def _prep_in_maps(x, U, V, bias):
    import ml_dtypes

    bf = ml_dtypes.bfloat16
    x = np.asarray(x, dtype=np.float32).reshape(TOK, D).astype(bf)
    U = np.asarray(U, dtype=np.float32).astype(bf)
    V = np.asarray(V, dtype=np.float32).astype(bf)

    # xh[tc, c, p, th*4096 + vj*512 + t] = x[tq*2048 + tc*TC + th*512 + t,
    #                                        c*1024 + vj*128 + p]
    # (th-major so stage 1 consumes the tile strictly left to right)
    xhs = []
    for tq in range(4):
        shard = x[tq * TQ : (tq + 1) * TQ]  # [2048, 4096]
        xh = shard.reshape(NCHUNK, 2, 512, NB, 8, 128).transpose(0, 3, 5, 1, 4, 2)
        xhs.append(np.ascontiguousarray(xh).reshape(NCHUNK, NB, 128, 8 * TC))

    vts, uts = [], []
    for og in range(2):
        Vg = V[og * 2 : og * 2 + 2]  # [2, 4, 256, 1024]
        vt = Vg.reshape(2, NB, 256, 8, 128).transpose(0, 1, 4, 3, 2)
        vts.append(np.ascontiguousarray(vt).reshape(8, 128, 2048))
        Ug = U[og * 2 : og * 2 + 2]  # [2, 4, 1024, 256]
        ut = Ug.reshape(2, NB, 1024, 2, 128).transpose(0, 1, 4, 3, 2)
        uts.append(np.ascontiguousarray(ut).reshape(8, 128, 2048))

    in_maps = []
    for g in range(NCORES):
        og, tq = g // 4, g % 4
        in_maps.append({"xh": xhs[tq], "vt": vts[og], "ut": uts[og]})
    return in_maps


def _assemble(results, bias):
    y = np.empty((TOK, D), dtype=np.float32)
    for g in range(NCORES):
        og, tq = g // 4, g % 4
        y[tq * TQ : (tq + 1) * TQ, og * 2048 : (og + 1) * 2048] = results[g]["y"].astype(
            np.float32
        )
    y = y.reshape(TOK // 2048, 2048, NB, BS)
    y += np.asarray(bias, dtype=np.float32)[None, None, :, :]
    return y.reshape(4, 2048, D)


def run_with_options(inputs, trace=False, **kw):
    from concourse.bass_utils import run_bass_kernel_spmd

    nc = _build_nc()
    in_maps = _prep_in_maps(**inputs)
    res = run_bass_kernel_spmd(nc, in_maps, core_ids=list(range(NCORES)), trace=trace, **kw)
    return _assemble(res.results, inputs["bias"]), res


def kernel(x, U, V, bias):
    out, _ = run_with_options({"x": x, "U": U, "V": V, "bias": bias})
    return out
